# revision 37
# baseline (speedup 1.0000x reference)
"""Allegro-style GNN message passing on 8 TRN2 NeuronCores.

Strategy:
- Host: shard edges by SENDER node range (1024 nodes/core) -> sender
  segment-sums are fully core-local (no cross-core collectives).
- Within a core, group edges by 128-node sender windows; pad each
  (core, window) group to a common K_WIN with dummy edges (d=2 -> u=0 ->
  zero contribution). One-hot matmuls on TensorE do segment-sum
  (scatter) and the gather-back.
- The one-hot matrices are GENERATED ON DEVICE (batched DVE is_equal
  against an iota row, DMA-xbar transpose for the gather side) from
  [128, T] index planes; sender attrs are gathered on device through
  the same one-hot matmuls. Only ~1.7 MB/core ships per call (vs ~19 MB
  with host-built one-hots).
- The axon execution path costs ~30-100us PER INSTRUCTION regardless of
  size (measured: DVE ~37us, PE ~30us, Act ~47us, DMA ~100-130us, plus
  ~50us per engine switch; no cross-engine dispatch parallelism), so
  everything batchable is batched: whole-window one-hot generation,
  whole-window DMA-xbar transposes (ohg/xw/w1/eo3 in ONE InstDmaTranspose
  each instead of one per 128-edge tile; xbar only supports short->tall,
  so the 128->16 partition besu/fb1 transposes stay on PE), gather-back
  matmuls packed 4-per-PSUM-bank-pair with one wide copy, sender AND
  receiver attrs gathered host-side and shipped as one [32, EP] blob,
  4D-broadcast products, and hoisted ytil/V10/fb products.
- Layer algebra: Y[:,0] == 1, so layer-1 only needs a 16-wide
  segment-sum of w1; W_lsh[1] output is dead; V1 is only needed at
  component 0 => contraction with Ytil = Y * W_lsh[0][:,0].
- Receiver scatter: node id = hi*128+lo; per edge-tile matmul with lo
  one-hot lhsT and (hi one-hot * edge_out) rhs accumulates [128,64]
  partials in PSUM; host sums the 8 per-core partials (the unshard).
- 1/sqrt(AVG_NEIGH) and the 1/sqrt(2) residual scales are folded into
  weights on the host.
"""
import math
import sys

import numpy as np

sys.path.insert(0, "/opt/trn_rl_repo")

import ml_dtypes  # noqa: E402

BF16 = ml_dtypes.bfloat16
SIM_SILU = False   # CoreSim lacks Silu; emulate with Sigmoid*z when set
IND_GATHER = False  # gather-backs via indirect DMA (False: PE one-hot matmuls)

N, E, MUL, H, F = 8192, 131072, 16, 256, 16
NB = 8
P6 = 6
INV = 1.0 / math.sqrt(16.0)
NC = 8
NPC = N // NC          # nodes per core
WIN = 128
NW = NPC // WIN        # windows per core
RWIN = N // WIN        # 64 receiver windows
SQ = math.sqrt(0.5)

# wblob row layout (all [*, H] bf16).  we0x rows (64, partition-aligned
# to match attrs_all): 0 zero (u row), 1:9 bessel, 9:32 zero,
# 32:48 sender, 48:64 receiver.
R_WE0, R_WE1 = 0, 64
R_WLY1 = [320, 592]
R_WLY2 = [864, 1120]
R_END = 1376
# fblob columns: [be0(2), be1(2), bly1_0(2), bly2_0(2), bly1_1(2),
#                 bly2_1(2), wcol(16)]
C_BE0, C_BE1 = 0, 2
C_BLY1 = [4, 8]
C_BLY2 = [6, 10]
C_WCOL = 12


def _host_shard(node_attrs, vectors, senders, receivers):
    """Group edges by (core, sender-window); pad to common K_WIN."""
    core = senders // NPC
    win = (senders % NPC) // WIN
    order = np.argsort(core * NW + win, kind="stable")
    key = (core * NW + win)[order]
    counts = np.bincount(key, minlength=NC * NW)
    kwin = int(((counts.max() + 127) // 128) * 128)
    starts = np.zeros(NC * NW + 1, np.int64)
    np.cumsum(counts, out=starts[1:])

    EP = NW * kwin
    shards = []
    for c in range(NC):
        vec = np.zeros((EP, 3), np.float32)
        vec[:, 0] = 2.0
        sra = np.zeros((EP, 2 * F), np.float32)
        sl = np.zeros(EP, np.int64)    # sender local-in-window
        rg = np.zeros(EP, np.int64)    # receiver global
        for w in range(NW):
            g = c * NW + w
            eid = order[starts[g]:starts[g + 1]]
            o = w * kwin
            n_e = len(eid)
            vec[o:o + n_e] = vectors[eid]
            sra[o:o + n_e, :F] = node_attrs[senders[eid]]
            sra[o:o + n_e, F:] = node_attrs[receivers[eid]]
            sl[o:o + n_e] = senders[eid] - (c * NPC + w * WIN)
            rg[o:o + n_e] = receivers[eid]
        shards.append((vec, sra, sl, rg))
    return kwin, shards


def _pack_core(kwin, vec, sra, sl, rg):
    """Per-core device arrays: index planes + endpoint attrs."""
    EP = NW * kwin
    T_ALL = EP // 128
    # plane layout: edge e = t*128 + p  ->  [128, T_ALL]
    edat = np.ascontiguousarray(
        vec.reshape(T_ALL, 128, 3).transpose(2, 1, 0)).astype(np.float32)
    eidx = np.zeros((3, 128, T_ALL), np.float32)
    eidx[0] = sl.reshape(T_ALL, 128).T
    eidx[1] = (rg % 128).reshape(T_ALL, 128).T
    eidx[2] = (rg // 128).reshape(T_ALL, 128).T
    srat = np.ascontiguousarray(sra.T).astype(BF16)           # [32, EP]
    slq = np.ascontiguousarray(sl.reshape(T_ALL, 128).T).astype(np.int32)
    return dict(edat=edat, eidx=eidx.astype(BF16), srat=srat, slq=slq)


def _prep_weights(i):
    """Fold INV and residual 1/sqrt(2) scales into weights; pack blobs."""
    wb = np.zeros((R_END, H), np.float32)
    wb[1:9] = i["W_e0"][0:8]          # bessel rows; row 0 stays zero (u)
    wb[32:48] = i["W_e0"][8:24]       # sender rows
    wb[48:64] = i["W_e0"][24:40]      # receiver rows
    wb[R_WE1:R_WE1 + 256] = i["W_e1"]
    wly1_1 = i["W_ly1"][1].copy()
    wly1_1[:H] *= SQ                                          # x1 = sq*x1'
    wb[R_WLY1[0]:R_WLY1[0] + 272] = i["W_ly1"][0]
    wb[R_WLY1[1]:R_WLY1[1] + 272] = wly1_1
    wb[R_WLY2[0]:R_WLY2[0] + 256] = i["W_ly2"][0]
    wb[R_WLY2[1]:R_WLY2[1] + 256] = i["W_ly2"][1]
    ws = np.zeros((H, 49), np.float32)
    ws[:, 0:16] = i["W_v0"]
    ws[:, 16:32] = i["W_lw"][0] * INV
    ws[:, 32:48] = i["W_lw"][1] * INV * SQ
    ws[:, 48:49] = i["W_out"] * INV * 0.5                     # x2 = .5*x2'
    fb = np.zeros((128, 28), np.float32)
    for j, b in enumerate([i["b_e0"], i["b_e1"], i["b_ly1"][0],
                           i["b_ly2"][0], i["b_ly1"][1], i["b_ly2"][1]]):
        fb[:, 2 * j] = b[:128]
        fb[:, 2 * j + 1] = b[128:]
    fb[:, C_WCOL:C_WCOL + 16] = i["W_lsh"][0][:, 0][None, :]
    return dict(wblob=wb.astype(BF16), wsmall=ws.astype(BF16),
                fblob=fb.astype(np.float32))


_CAP_SKIP = {"InstEventSemaphore", "InstBranch", "InstNop",
             "InstCollectiveCompute"}
_CAP_LIMITS = {}


def _split_waits(nc, mybir, mk_carrier, limit=1):
    """Walrus codegen allows only 1 embedded sem-wait on compute
    instructions.  For each instruction with more, strip the extras onto
    freshly created same-engine carrier instructions inserted directly
    before it (engines are in-order, so this preserves semantics)."""
    f = nc.m.functions[0]
    made = 0
    # find blocks that carriers get appended to, to strip later
    for bb in f.blocks:
        insts = list(bb.instructions)
        plan = []          # (index, [carrier insts])
        for i, inst in enumerate(insts):
            tname = type(inst).__name__
            si = inst.sync_info
            nwait = len(si.on_wait) if (si and si.on_wait) else 0
            lim = _CAP_LIMITS.get(tname, limit)
            if tname in _CAP_SKIP or nwait <= lim:
                continue
            waits = list(si.on_wait)
            extras, keep = waits[:-lim], waits[-lim:]
            carriers = []
            for wt in extras:
                ci = mk_carrier(inst.engine)
                if ci is None:
                    keep.insert(0, wt)
                    continue
                ci.sync_info = mybir.SyncInfo(on_wait=[wt], on_update=[])
                carriers.append(ci)
                made += 1
            inst.sync_info = mybir.SyncInfo(on_wait=keep,
                                            on_update=si.on_update)
            if carriers:
                plan.append((i, carriers))
        if plan:
            new = []
            pmap = dict(plan)
            for i, inst in enumerate(insts):
                if i in pmap:
                    new.extend(pmap[i])
                new.append(inst)
            bb.instructions = new
    return made


def build_graph(kwin, w):
    from concourse import bass, mybir
    from concourse.bass import IndirectOffsetOnAxis
    from concourse.masks import make_identity
    from concourse.tile import TileContext

    EP = NW * kwin
    T_ALL = EP // 128
    T_W = kwin // 128
    NCH = (kwin + 511) // 512      # free chunks per window

    f32 = mybir.dt.float32
    bf16 = mybir.dt.bfloat16
    AX = mybir.AxisListType.X
    OP = mybir.AluOpType
    AF = mybir.ActivationFunctionType

    nc = bass.Bass()
    carrier_sem_cm = nc.semaphore("carrier_sem")
    carrier_sem = carrier_sem_cm.__enter__()
    dp = nc.declare_dram_parameter
    d_edat = dp("edat", [3, 128, T_ALL], f32, isOutput=False)
    d_eidx = dp("eidx", [3, 128, T_ALL], bf16, isOutput=False)
    d_slq = dp("slq", [128, T_ALL], mybir.dt.int32, isOutput=False)
    d_srat = dp("srat", [32, EP], bf16, isOutput=False)
    # DRAM bounce buffers for indirect gathers: declared as outputs so they
    # live in plain writable external DRAM (internal DRAM tiles fault the
    # dynamic-AP path at runtime)
    d_wy0 = dp("wy0", [128, 256], bf16, isOutput=True)
    d_wy1 = dp("wy1", [128, MUL], bf16, isOutput=True)
    # weights ride inside the NEFF as constants -- they never transfer
    # with the per-call inputs
    d_wblob = nc.inline_tensor(np.asarray(w["wblob"]), name="wblob")
    d_wsmall = nc.inline_tensor(np.asarray(w["wsmall"]), name="wsmall")
    d_fblob = nc.inline_tensor(np.asarray(w["fblob"]), name="fblob")
    d_out = dp("out", [128, RWIN], f32, isOutput=True)

    with TileContext(nc) as tc:
        with (
            tc.tile_pool(name="glob", bufs=1) as gp,
            tc.tile_pool(name="wgt", bufs=1) as wp,
            tc.tile_pool(name="win", bufs=1) as wnp,
            tc.tile_pool(name="big", bufs=1) as bgp,
            tc.tile_pool(name="sml", bufs=3) as sp,
            tc.tile_pool(name="ps_mlp", bufs=2, space="PSUM") as pmlp,
            tc.tile_pool(name="ps_acc", bufs=1, space="PSUM") as pacc,
            tc.tile_pool(name="ps_sml", bufs=1, space="PSUM") as psml,
            tc.tile_pool(name="ps_rcv", bufs=1, space="PSUM") as prcv,
        ):
            # ---------------- weights to SBUF ----------------
            def ldw(r0, r1, tag):
                t = wp.tile([r1 - r0, H], bf16, tag=tag)
                nc.sync.dma_start(out=t[:], in_=d_wblob[r0:r1, :])
                return t

            def ldw2(r0, tag):
                # [256, H] -> [128, 2, H] k-chunked
                t = wp.tile([128, 2, H], bf16, tag=tag)
                for kc in range(2):
                    nc.sync.dma_start(
                        out=t[:, kc, :],
                        in_=d_wblob[r0 + kc * 128:r0 + (kc + 1) * 128, :])
                return t
            we0x = ldw(0, 64, "we0x")
            we1 = ldw2(R_WE1, "we1")
            wly1 = [ldw2(R_WLY1[0], "wly1_0"), ldw2(R_WLY1[1], "wly1_1")]
            wly1fb = [ldw(R_WLY1[0] + 256, R_WLY1[0] + 272, "wly1fb_0"),
                      ldw(R_WLY1[1] + 256, R_WLY1[1] + 272, "wly1fb_1")]
            wly2 = [ldw2(R_WLY2[0], "wly2_0"), ldw2(R_WLY2[1], "wly2_1")]
            wsm = wp.tile([128, 2, 49], bf16, tag="wsm")
            for kc in range(2):
                nc.sync.dma_start(out=wsm[:, kc, :],
                                  in_=d_wsmall[kc * 128:(kc + 1) * 128, :])
            fbt = wp.tile([128, 28], f32, tag="fblob")
            nc.sync.dma_start(out=fbt[:], in_=d_fblob[:])
            # attrs_all rows (partition-aligned starts): 0 u-fm,
            # 1:9 bessel-fm, 32:48 sender attrs, 48:64 receiver attrs --
            # the single e0 rhs (K=64); rows 9:32 zeroed (zero weights)
            attrs_all = gp.tile([64, EP], bf16)
            nc.vector.memset(attrs_all[:], 0.0)
            nc.sync.dma_start(out=attrs_all[32:64, :], in_=d_srat[:])

            bias = {
                "be0": [fbt[:, C_BE0 + h:C_BE0 + h + 1] for h in range(2)],
                "be1": [fbt[:, C_BE1 + h:C_BE1 + h + 1] for h in range(2)],
                "bly1": [[fbt[:, c + h:c + h + 1] for h in range(2)]
                         for c in C_BLY1],
                "bly2": [[fbt[:, c + h:c + h + 1] for h in range(2)]
                         for c in C_BLY2],
            }
            wcol = fbt[:, C_WCOL:C_WCOL + 16]

            ident = wp.tile([128, 128], f32, tag="ident")
            make_identity(nc, ident[:])
            ones_bf = wp.tile([1, 128], bf16, tag="ones")
            nc.vector.memset(ones_bf[:], 1.0)
            iota_f = wp.tile([128, 128], bf16, tag="iotaf")
            nc.gpsimd.iota(iota_f[:], pattern=[[1, 128]], base=0,
                           channel_multiplier=0,
                           allow_small_or_imprecise_dtypes=True)

            # ---------------- edge-scalar stage (planes [128,T_ALL]) ----
            vx = gp.tile([128, T_ALL], f32)
            vy = gp.tile([128, T_ALL], f32)
            vz = gp.tile([128, T_ALL], f32)
            nc.gpsimd.dma_start(out=vx[:], in_=d_edat[0])
            nc.gpsimd.dma_start(out=vy[:], in_=d_edat[1])
            nc.gpsimd.dma_start(out=vz[:], in_=d_edat[2])
            sl_pl = gp.tile([128, T_ALL], bf16)
            rq_pl = gp.tile([128, T_ALL], bf16)
            rw_pl = gp.tile([128, T_ALL], bf16)
            nc.gpsimd.dma_start(out=sl_pl[:], in_=d_eidx[0])
            nc.gpsimd.dma_start(out=rq_pl[:], in_=d_eidx[1])
            nc.gpsimd.dma_start(out=rw_pl[:], in_=d_eidx[2])
            slq = gp.tile([128, T_ALL], mybir.dt.int32)
            nc.gpsimd.dma_start(out=slq[:], in_=d_slq[:])
            ta = gp.tile([128, T_ALL], f32)
            tb = gp.tile([128, T_ALL], f32)
            tt = nc.vector.tensor_tensor
            ts = nc.vector.tensor_scalar
            act = nc.scalar.activation

            def silu_act(out, ps_in, bias_ap):
                if not SIM_SILU:
                    act(out=out, in_=ps_in, func=AF.Silu, bias=bias_ap)
                else:
                    pp = ps_in.shape[0]
                    sg = bgp.tile([128, 512], f32, tag="simsilu")
                    zz_ = bgp.tile([128, 512], f32, tag="simsilu2")
                    cw_ = ps_in.shape[-1]
                    act(out=sg[:pp, :cw_], in_=ps_in, func=AF.Sigmoid,
                        bias=bias_ap)
                    nc.vector.tensor_scalar(out=zz_[:pp, :cw_], in0=ps_in,
                                            scalar1=bias_ap, scalar2=None,
                                            op0=OP.add)
                    nc.vector.tensor_mul(out=out, in0=sg[:pp, :cw_],
                                         in1=zz_[:pp, :cw_])
            d_pl = gp.tile([128, T_ALL], f32)
            nc.vector.tensor_mul(out=ta[:], in0=vx[:], in1=vx[:])
            nc.vector.tensor_mul(out=tb[:], in0=vy[:], in1=vy[:])
            nc.vector.tensor_add(out=ta[:], in0=ta[:], in1=tb[:])
            nc.vector.tensor_mul(out=tb[:], in0=vz[:], in1=vz[:])
            nc.vector.tensor_add(out=ta[:], in0=ta[:], in1=tb[:])
            act(out=d_pl[:], in_=ta[:], func=AF.Sqrt)
            rinv = gp.tile([128, T_ALL], f32)
            nc.vector.reciprocal(out=rinv[:], in_=d_pl[:])
            ux = gp.tile([128, T_ALL], f32)
            uy = gp.tile([128, T_ALL], f32)
            uz = gp.tile([128, T_ALL], f32)
            nc.vector.tensor_mul(out=ux[:], in0=vx[:], in1=rinv[:])
            nc.vector.tensor_mul(out=uy[:], in0=vy[:], in1=rinv[:])
            nc.vector.tensor_mul(out=uz[:], in0=vz[:], in1=rinv[:])

            # besu9: col 0 envelope u, cols 1:9 bessel (transposed together)
            besu = gp.tile([128, T_ALL, 9], f32)
            # envelope u = 1 + d^6*(-28 + 48d - 21d^2), zero for d >= 1
            nc.vector.tensor_mul(out=ta[:], in0=d_pl[:], in1=d_pl[:])   # d2
            nc.vector.tensor_mul(out=tb[:], in0=ta[:], in1=d_pl[:])     # d3
            nc.vector.tensor_mul(out=tb[:], in0=tb[:], in1=tb[:])       # d6
            ts(out=ta[:], in0=ta[:], scalar1=-21.0, scalar2=None, op0=OP.mult)
            tc_q = gp.tile([128, T_ALL], f32)
            ts(out=tc_q[:], in0=d_pl[:], scalar1=48.0, scalar2=-28.0,
               op0=OP.mult, op1=OP.add)
            nc.vector.tensor_add(out=ta[:], in0=ta[:], in1=tc_q[:])
            nc.vector.tensor_mul(out=tb[:], in0=tb[:], in1=ta[:])
            ts(out=tb[:], in0=tb[:], scalar1=1.0, scalar2=None, op0=OP.add)
            ts(out=ta[:], in0=d_pl[:], scalar1=1.0, scalar2=None,
               op0=OP.is_lt)
            nc.vector.tensor_mul(out=besu[:, :, 0], in0=tb[:], in1=ta[:])

            # spherical harmonics Y [128, T_ALL, 16] f32
            Yt = gp.tile([128, T_ALL, 16], f32)
            s3 = 3.0 ** 0.5; s5 = 5.0 ** 0.5; s15 = 15.0 ** 0.5
            s7 = 7.0 ** 0.5
            c33 = (35.0 / 8.0) ** 0.5; c32 = 105.0 ** 0.5
            c31 = (21.0 / 8.0) ** 0.5
            xx = gp.tile([128, T_ALL], f32)
            yy = gp.tile([128, T_ALL], f32)
            zz = gp.tile([128, T_ALL], f32)
            xy = gp.tile([128, T_ALL], f32)
            nc.vector.tensor_mul(out=xx[:], in0=ux[:], in1=ux[:])
            nc.vector.tensor_mul(out=yy[:], in0=uy[:], in1=uy[:])
            nc.vector.tensor_mul(out=zz[:], in0=uz[:], in1=uz[:])
            nc.vector.tensor_mul(out=xy[:], in0=ux[:], in1=uy[:])
            ts(out=Yt[:, :, 0], in0=ux[:], scalar1=0.0, scalar2=1.0,
               op0=OP.mult, op1=OP.add)
            ts(out=Yt[:, :, 1], in0=ux[:], scalar1=s3, scalar2=None,
               op0=OP.mult)
            ts(out=Yt[:, :, 2], in0=uy[:], scalar1=s3, scalar2=None,
               op0=OP.mult)
            ts(out=Yt[:, :, 3], in0=uz[:], scalar1=s3, scalar2=None,
               op0=OP.mult)
            ts(out=Yt[:, :, 4], in0=xy[:], scalar1=s15, scalar2=None,
               op0=OP.mult)
            nc.vector.tensor_mul(out=ta[:], in0=uy[:], in1=uz[:])
            ts(out=Yt[:, :, 5], in0=ta[:], scalar1=s15, scalar2=None,
               op0=OP.mult)
            ts(out=Yt[:, :, 6], in0=zz[:], scalar1=1.5 * s5,
               scalar2=-0.5 * s5, op0=OP.mult, op1=OP.add)
            nc.vector.tensor_mul(out=tb[:], in0=ux[:], in1=uz[:])
            ts(out=Yt[:, :, 7], in0=tb[:], scalar1=s15, scalar2=None,
               op0=OP.mult)
            xmy = gp.tile([128, T_ALL], f32)
            nc.vector.tensor_sub(out=xmy[:], in0=xx[:], in1=yy[:])
            ts(out=Yt[:, :, 8], in0=xmy[:], scalar1=0.5 * s15, scalar2=None,
               op0=OP.mult)
            # Y9 = c33*y*(3xx-yy)
            ts(out=ta[:], in0=xx[:], scalar1=3.0, scalar2=None, op0=OP.mult)
            nc.vector.tensor_sub(out=ta[:], in0=ta[:], in1=yy[:])
            nc.vector.tensor_mul(out=ta[:], in0=ta[:], in1=uy[:])
            ts(out=Yt[:, :, 9], in0=ta[:], scalar1=c33, scalar2=None,
               op0=OP.mult)
            # Y10 = c32*x*y*z
            nc.vector.tensor_mul(out=ta[:], in0=xy[:], in1=uz[:])
            ts(out=Yt[:, :, 10], in0=ta[:], scalar1=c32, scalar2=None,
               op0=OP.mult)
            # Y11/Y13: c31*{y,x}*(5zz-1)
            ts(out=ta[:], in0=zz[:], scalar1=5.0, scalar2=-1.0,
               op0=OP.mult, op1=OP.add)
            nc.vector.tensor_mul(out=tb[:], in0=ta[:], in1=uy[:])
            ts(out=Yt[:, :, 11], in0=tb[:], scalar1=c31, scalar2=None,
               op0=OP.mult)
            nc.vector.tensor_mul(out=tb[:], in0=ta[:], in1=ux[:])
            ts(out=Yt[:, :, 13], in0=tb[:], scalar1=c31, scalar2=None,
               op0=OP.mult)
            # Y12 = 2.5*s7*z^3 - 1.5*s7*z
            nc.vector.tensor_mul(out=ta[:], in0=zz[:], in1=uz[:])
            ts(out=ta[:], in0=ta[:], scalar1=2.5 * s7, scalar2=None,
               op0=OP.mult)
            ts(out=tb[:], in0=uz[:], scalar1=1.5 * s7, scalar2=None,
               op0=OP.mult)
            nc.vector.tensor_sub(out=Yt[:, :, 12], in0=ta[:], in1=tb[:])
            # Y14 = 0.5*c32*z*(xx-yy)
            nc.vector.tensor_mul(out=ta[:], in0=xmy[:], in1=uz[:])
            ts(out=Yt[:, :, 14], in0=ta[:], scalar1=0.5 * c32, scalar2=None,
               op0=OP.mult)
            # Y15 = c33*x*(xx-3yy)
            ts(out=ta[:], in0=yy[:], scalar1=3.0, scalar2=None, op0=OP.mult)
            nc.vector.tensor_sub(out=ta[:], in0=xx[:], in1=ta[:])
            nc.vector.tensor_mul(out=ta[:], in0=ta[:], in1=ux[:])
            ts(out=Yt[:, :, 15], in0=ta[:], scalar1=c33, scalar2=None,
               op0=OP.mult)

            # bessel (range-reduced) -> besu cols 0:8
            rs = gp.tile([128, T_ALL], f32)
            ts(out=rs[:], in0=rinv[:], scalar1=math.sqrt(2.0), scalar2=None,
               op0=OP.mult)
            mi = gp.tile([128, T_ALL], mybir.dt.int32)
            for k in range(1, NB + 1):
                ts(out=ta[:], in0=d_pl[:], scalar1=0.5 * k, scalar2=None,
                   op0=OP.mult)
                nc.vector.tensor_copy(out=mi[:], in_=ta[:])
                nc.vector.tensor_copy(out=tb[:], in_=mi[:])
                nc.vector.tensor_sub(out=ta[:], in0=ta[:], in1=tb[:])
                # ta = frac in (-0.5, 1) whether the cast rounds or truncates
                ts(out=tb[:], in0=ta[:], scalar1=0.5, scalar2=None,
                   op0=OP.is_gt)
                nc.vector.tensor_sub(out=ta[:], in0=ta[:], in1=tb[:])
                act(out=ta[:], in_=ta[:], func=AF.Sin, scale=2.0 * math.pi)
                nc.vector.tensor_mul(out=besu[:, :, k], in0=ta[:],
                                      in1=rs[:])


            # ytil = Y * wcol, hoisted out of the gather loop
            ytil_g = gp.tile([128, T_ALL, 16], f32)
            nc.vector.tensor_mul(
                out=ytil_g[:], in0=Yt[:],
                in1=wcol[:, None, :].to_broadcast([128, T_ALL, 16]))

            # bes/u rows 0:9 of attrs_all via per-tile PE transposes
            for tt_ in range(T_ALL):
                pst = psml.tile([16, 128], f32, space="PSUM", tag="sml")
                nc.tensor.transpose(out=pst[0:9, :], in_=besu[:, tt_, :],
                                    identity=ident[:])
                nc.vector.tensor_copy(
                    out=attrs_all[0:9, tt_ * 128:(tt_ + 1) * 128],
                    in_=pst[0:9, :])

            # ---------------- persistent receiver accumulator ----------
            ps_rcv = prcv.tile([128, RWIN], f32, space="PSUM")

            # ---------------- window loop ----------------
            for w in range(NW):
                t0 = w * T_W
                wsl = slice(w * kwin, (w + 1) * kwin)
                # one-hots: batched is_equal against the iota row
                ohs = wnp.tile([128, T_W, 128], bf16)
                ohg = wnp.tile([128, T_W, 128], bf16)
                rqt = wnp.tile([128, T_W, 128], bf16)
                rwt = wnp.tile([128, T_W, RWIN], bf16)
                tt(out=ohs[:],
                   in0=iota_f[:, None, :].to_broadcast([128, T_W, 128]),
                   in1=sl_pl[:, t0:t0 + T_W, None].to_broadcast(
                       [128, T_W, 128]), op=OP.is_equal)
                tt(out=rqt[:],
                   in0=iota_f[:, None, :].to_broadcast([128, T_W, 128]),
                   in1=rq_pl[:, t0:t0 + T_W, None].to_broadcast(
                       [128, T_W, 128]), op=OP.is_equal)
                tt(out=rwt[:],
                   in0=iota_f[:, None, 0:RWIN].to_broadcast(
                       [128, T_W, RWIN]),
                   in1=rw_pl[:, t0:t0 + T_W, None].to_broadcast(
                       [128, T_W, RWIN]), op=OP.is_equal)
                nc.sync.dma_start_transpose(
                    out=ohg[:], in_=ohs[:].rearrange("p a b -> p (a b)"))
                ohg_v = ohg[:].rearrange("p a b -> p (a b)")
                # broadcast u row -> [128, kwin] bf16
                ubc = bgp.tile([128, kwin], bf16)
                for ch in range(NCH):
                    c0 = ch * 512
                    c1 = min(kwin, c0 + 512)
                    psu = pmlp.tile([128, 512], f32, space="PSUM", tag="mlp")
                    nc.tensor.matmul(out=psu[:, :c1 - c0], lhsT=ones_bf[:],
                                     rhs=attrs_all[0:1, wsl][:, c0:c1],
                                     start=True, stop=True)
                    nc.vector.tensor_copy(out=ubc[:, c0:c1],
                                          in_=psu[:, :c1 - c0])

                # ---- edge MLP: x0 = u*silu(e1(silu(e0(bes,attrs)))) ----
                x0 = bgp.tile([128, 2, kwin], bf16)
                th = bgp.tile([128, 2, kwin], bf16)
                for ch in range(NCH):
                    c0 = ch * 512
                    c1 = min(kwin, c0 + 512)
                    cw = c1 - c0
                    for hc in range(2):
                        hs = slice(hc * 128, (hc + 1) * 128)
                        ps = pmlp.tile([128, 512], f32, space="PSUM", tag="mlp")
                        nc.tensor.matmul(out=ps[:, :cw], lhsT=we0x[:, hs],
                                         rhs=attrs_all[:, wsl][:, c0:c1],
                                         start=True, stop=True)
                        silu_act(th[:, hc, c0:c1], ps[:, :cw],
                                 bias["be0"][hc])
                for ch in range(NCH):
                    c0 = ch * 512
                    c1 = min(kwin, c0 + 512)
                    cw = c1 - c0
                    for hc in range(2):
                        hs = slice(hc * 128, (hc + 1) * 128)
                        ps = pmlp.tile([128, 512], f32, space="PSUM", tag="mlp")
                        for kc in range(2):
                            nc.tensor.matmul(out=ps[:, :cw],
                                             lhsT=we1[:, kc, hs],
                                             rhs=th[:, kc, c0:c1],
                                             start=(kc == 0), stop=(kc == 1))
                        silu_act(x0[:, hc, c0:c1], ps[:, :cw],
                                 bias["be1"][hc])
                nc.vector.tensor_mul(
                    out=x0[:], in0=x0[:],
                    in1=ubc[:, None, :].to_broadcast([128, 2, kwin]))

                # ---- xv|w0 feature-major, DMA-transposed to edge-major ----
                xw_fm = wnp.tile([32, kwin], bf16)
                for ch in range(NCH):
                    c0 = ch * 512
                    c1 = min(kwin, c0 + 512)
                    px = pmlp.tile([128, 512], f32, space="PSUM", tag="mlp")
                    for kc in range(2):
                        nc.tensor.matmul(out=px[0:32, :c1 - c0],
                                         lhsT=wsm[:, kc, 0:32],
                                         rhs=x0[:, kc, c0:c1],
                                         start=(kc == 0), stop=(kc == 1))
                    nc.vector.tensor_copy(out=xw_fm[:, c0:c1],
                                          in_=px[0:32, :c1 - c0])
                xw = wnp.tile([128, T_W, 32], bf16)
                nc.sync.dma_start_transpose(out=xw[:], in_=xw_fm[:])

                # ---- layer-0 scatter: wY[n, m*16+i] ----
                v2w = wnp.tile([128, T_W, MUL, 16], bf16)
                nc.vector.tensor_mul(
                    out=v2w[:],
                    in0=xw[:, :, 16:32, None].to_broadcast(
                        [128, T_W, MUL, 16]),
                    in1=Yt[:, t0:t0 + T_W, None, :].to_broadcast(
                        [128, T_W, MUL, 16]))
                ps_acc = pacc.tile([128, 256], f32, space="PSUM", tag="acc")
                for t in range(T_W):
                    nc.tensor.matmul(
                        out=ps_acc[:],
                        lhsT=ohs[:, t, :],
                        rhs=v2w[:, t].rearrange("p a b -> p (a b)"),
                        start=(t == 0), stop=(t == T_W - 1))
                wY = wnp.tile([128, 256], bf16)
                nc.vector.tensor_copy(out=wY[:], in_=ps_acc[:])

                # ---- gather-back via indirect DMA through a DRAM bounce ----
                wYe = wnp.tile([128, T_W, 256], bf16)
                if IND_GATHER:
                    nc.sync.dma_start(out=d_wy0[:], in_=wY[:])
                    nc.gpsimd.indirect_dma_start(
                        out=wYe[:], in_=d_wy0[:],
                        in_offset=IndirectOffsetOnAxis(
                            ap=slq[:, t0:t0 + T_W], axis=0),
                        out_offset=None)
                else:
                    for t4 in range(0, T_W, 2):
                        t5 = min(T_W, t4 + 2)
                        pg = pmlp.tile([128, 512], f32, space="PSUM",
                                       tag="mlp")
                        for t in range(t4, t5):
                            i4 = t - t4
                            nc.tensor.matmul(
                                out=pg[:, i4 * 256:(i4 + 1) * 256],
                                lhsT=ohg[:, t, :], rhs=wY[:],
                                start=True, stop=True)
                        nc.vector.tensor_copy(
                            out=wYe[:, t4:t5, :].rearrange(
                                "p a b -> p (a b)"),
                            in_=pg[:, :(t5 - t4) * 256])
                prodw = wnp.tile([128, T_W, MUL, 16], bf16)
                nc.vector.tensor_mul(
                    out=prodw[:],
                    in0=wYe[:].rearrange("p t (a b) -> p t a b", b=16),
                    in1=ytil_g[:, t0:t0 + T_W, None, :].to_broadcast(
                        [128, T_W, MUL, 16]))
                Sw = wnp.tile([128, T_W, MUL], f32)
                nc.vector.reduce_sum(out=Sw[:, :, :, None], in_=prodw[:],
                                     axis=AX)
                V10w = wnp.tile([128, T_W, MUL], f32)
                nc.vector.tensor_mul(out=V10w[:], in0=Sw[:],
                                     in1=xw[:, :, 0:16])
                # fb feature-major directly: wYe0_fm = wY[:,0::16]^T @ ohg,
                # times xv_fm (= xw_fm rows 0:16) -- no per-tile transposes
                fbfm = wnp.tile([MUL, kwin], bf16)
                for ch in range(NCH):
                    c0 = ch * 512
                    c1 = min(kwin, c0 + 512)
                    pf = pmlp.tile([128, 512], f32, space="PSUM", tag="mlp")
                    nc.tensor.matmul(out=pf[0:MUL, :c1 - c0],
                                     lhsT=wY[:, 0:256:16],
                                     rhs=ohg_v[:, c0:c1],
                                     start=True, stop=True)
                    nc.vector.tensor_mul(out=fbfm[:, c0:c1],
                                         in0=pf[0:MUL, :c1 - c0],
                                         in1=xw_fm[0:16, c0:c1])

                # ---- layer-0 ly1/ly2 + residual -> x1 ----
                x1 = bgp.tile([128, 2, kwin], bf16)

                def mlp_block(xin, xout, wl1, wl1fb, bl1, wl2, bl2, fbrow,
                              resid_sq2):
                    ty = bgp.tile([128, 2, kwin], bf16)
                    for ch in range(NCH):
                        c0 = ch * 512
                        c1 = min(kwin, c0 + 512)
                        cw = c1 - c0
                        for hc in range(2):
                            hs = slice(hc * 128, (hc + 1) * 128)
                            ps = pmlp.tile([128, 512], f32, space="PSUM",
                                           tag="mlp")
                            for kc in range(2):
                                nc.tensor.matmul(out=ps[:, :cw],
                                                 lhsT=wl1[:, kc, hs],
                                                 rhs=xin[:, kc, c0:c1],
                                                 start=(kc == 0), stop=False)
                            nc.tensor.matmul(out=ps[:, :cw],
                                             lhsT=wl1fb[:, hs],
                                             rhs=fbrow[:, c0:c1],
                                             start=False, stop=True)
                            silu_act(ty[:, hc, c0:c1], ps[:, :cw], bl1[hc])
                    ty2 = bgp.tile([128, 2, kwin], bf16)
                    for ch in range(NCH):
                        c0 = ch * 512
                        c1 = min(kwin, c0 + 512)
                        cw = c1 - c0
                        for hc in range(2):
                            hs = slice(hc * 128, (hc + 1) * 128)
                            ps = pmlp.tile([128, 512], f32, space="PSUM",
                                           tag="mlp")
                            for kc in range(2):
                                nc.tensor.matmul(out=ps[:, :cw],
                                                 lhsT=wl2[:, kc, hs],
                                                 rhs=ty[:, kc, c0:c1],
                                                 start=(kc == 0),
                                                 stop=(kc == 1))
                            silu_act(ty2[:, hc, c0:c1], ps[:, :cw], bl2[hc])
                    # x_out' = x_in' + s * u * y   (s = 1 or sqrt(2))
                    nc.vector.tensor_mul(
                        out=ty2[:], in0=ty2[:],
                        in1=ubc[:, None, :].to_broadcast([128, 2, kwin]))
                    if resid_sq2:
                        ts(out=ty2[:], in0=ty2[:], scalar1=math.sqrt(2.0),
                           scalar2=None, op0=OP.mult)
                    nc.vector.tensor_add(out=xout[:], in0=xin[:],
                                         in1=ty2[:])

                mlp_block(x0, x1, wly1[0], wly1fb[0], bias["bly1"][0],
                          wly2[0], bias["bly2"][0], fbfm, False)

                # ---- layer 1: w1, 16-wide scatter/gather, feedback ----
                w1_fm = wnp.tile([MUL, kwin], bf16)
                for ch in range(NCH):
                    c0 = ch * 512
                    c1 = min(kwin, c0 + 512)
                    px = pmlp.tile([128, 512], f32, space="PSUM", tag="mlp")
                    for kc in range(2):
                        nc.tensor.matmul(out=px[0:MUL, :c1 - c0],
                                         lhsT=wsm[:, kc, 32:48],
                                         rhs=x1[:, kc, c0:c1],
                                         start=(kc == 0), stop=(kc == 1))
                    nc.vector.tensor_copy(out=w1_fm[:, c0:c1],
                                          in_=px[0:MUL, :c1 - c0])
                w1 = wnp.tile([128, T_W, MUL], bf16)
                nc.sync.dma_start_transpose(out=w1[:], in_=w1_fm[:])
                ps_a1 = pacc.tile([128, 256], f32, space="PSUM", tag="acc")
                for t in range(T_W):
                    nc.tensor.matmul(out=ps_a1[:, 0:MUL], lhsT=ohs[:, t, :],
                                     rhs=w1[:, t, :],
                                     start=(t == 0), stop=(t == T_W - 1))
                wY1 = wnp.tile([128, MUL], bf16)
                nc.vector.tensor_copy(out=wY1[:], in_=ps_a1[:, 0:MUL])
                w1e = wnp.tile([128, T_W, MUL], bf16)
                if IND_GATHER:
                    nc.sync.dma_start(out=d_wy1[:], in_=wY1[:])
                    nc.gpsimd.indirect_dma_start(
                        out=w1e[:], in_=d_wy1[:],
                        in_offset=IndirectOffsetOnAxis(
                            ap=slq[:, t0:t0 + T_W], axis=0),
                        out_offset=None)
                else:
                    pg1 = pmlp.tile([128, 512], f32, space="PSUM", tag="mlp")
                    for t in range(T_W):
                        nc.tensor.matmul(out=pg1[:, t * MUL:(t + 1) * MUL],
                                         lhsT=ohg[:, t, :], rhs=wY1[:],
                                         start=True, stop=True)
                    nc.vector.tensor_copy(
                        out=w1e[:].rearrange("p a b -> p (a b)"),
                        in_=pg1[:, :T_W * MUL])
                fb1w = wnp.tile([128, T_W, MUL], f32)
                nc.vector.tensor_mul(out=fb1w[:], in0=w1e[:], in1=V10w[:])
                fbfm1 = wnp.tile([MUL, kwin], bf16)
                for t in range(T_W):
                    pst = psml.tile([16, 128], f32, space="PSUM", tag="sml")
                    nc.tensor.transpose(out=pst[:], in_=fb1w[:, t, :],
                                        identity=ident[:])
                    nc.vector.tensor_copy(out=fbfm1[:, t * 128:(t + 1) * 128],
                                          in_=pst[:])

                # ---- layer-1 ly1/ly2 + residual -> x2 ----
                x2 = bgp.tile([128, 2, kwin], bf16)
                mlp_block(x1, x2, wly1[1], wly1fb[1], bias["bly1"][1],
                          wly2[1], bias["bly2"][1], fbfm1, True)

                # ---- edge out feature-major (row 0 of a 16-row tile so
                # the DMA-xbar transpose is legal), u folded in place ----
                eo16 = wnp.tile([16, kwin], bf16)
                nc.vector.memset(eo16[:], 0.0)
                for ch in range(NCH):
                    c0 = ch * 512
                    c1 = min(kwin, c0 + 512)
                    pf = pmlp.tile([128, 512], f32, space="PSUM", tag="mlp")
                    for kc in range(2):
                        nc.tensor.matmul(out=pf[0:1, :c1 - c0],
                                         lhsT=wsm[:, kc, 48:49],
                                         rhs=x2[:, kc, c0:c1],
                                         start=(kc == 0), stop=(kc == 1))
                    nc.vector.tensor_mul(out=eo16[0:1, c0:c1],
                                         in0=pf[0:1, :c1 - c0],
                                         in1=attrs_all[0:1, wsl][:, c0:c1])
                eo3 = wnp.tile([128, T_W, 16], bf16)
                nc.sync.dma_start_transpose(out=eo3[:], in_=eo16[:])
                mtw = wnp.tile([128, T_W, RWIN], bf16)
                nc.vector.tensor_mul(
                    out=mtw[:], in0=rwt[:],
                    in1=eo3[:, :, 0, None].to_broadcast([128, T_W, RWIN]))
                for t in range(T_W):
                    nc.tensor.matmul(out=ps_rcv[:], lhsT=rqt[:, t, :],
                                     rhs=mtw[:, t, :],
                                     start=(w == 0 and t == 0),
                                     stop=(w == NW - 1 and t == T_W - 1))

            out_sb = gp.tile([128, RWIN], f32)
            nc.vector.tensor_copy(out=out_sb[:], in_=ps_rcv[:])
            nc.sync.dma_start(out=d_out[:], in_=out_sb[:])

    ET = mybir.EngineType
    eng_map = {ET.DVE: nc.vector, ET.Activation: nc.scalar,
               ET.Pool: nc.gpsimd, ET.PE: nc.tensor, ET.SP: nc.sync}

    def mk_carrier(eng):
        be = eng_map.get(eng)
        if be is None:
            return None
        w = be.wait_ge(carrier_sem, 0)
        ci = w.ins if hasattr(w, "ins") else w
        # strip from whatever block it was appended to
        for bb in nc.m.functions[0].blocks:
            il = list(bb.instructions)
            if any(x is ci for x in il):
                bb.instructions = [x for x in il if x is not ci]
                break
        return ci

    made = _split_waits(nc, mybir, mk_carrier)
    print(f"split_waits: carriers={made}", flush=True)
    return nc


def make_in_maps(inputs):
    kwin, shards = _host_shard(inputs["node_attrs"], inputs["vectors"],
                               inputs["senders"], inputs["receivers"])
    in_maps = []
    for c in range(NC):
        m = _pack_core(kwin, *shards[c])
        in_maps.append({k: np.ascontiguousarray(v) for k, v in m.items()})
    return kwin, in_maps


def kernel(**inputs):
    inputs = {k: np.asarray(v) for k, v in inputs.items()}
    kwin, in_maps = make_in_maps(inputs)
    nc = build_graph(kwin, _prep_weights(inputs))
    from concourse.bass_utils import run_bass_kernel_spmd
    res = run_bass_kernel_spmd(nc, in_maps, core_ids=list(range(NC)))
    out = np.zeros((128, RWIN), np.float64)
    for r in res.results:
        out += np.asarray(r["out"], np.float64)
    # node n = hi*128 + lo stored at [lo, hi]
    return np.ascontiguousarray(out.T.reshape(N, 1)).astype(np.float32)

